# revision 13
# baseline (speedup 1.0000x reference)
"""Bass/Tile kernel for nn_DeepSeekBlock (MoE routing + MLA block), v2.

Data-parallel over batch: 1024 tokens/core on 8 cores. Per rep:
  Phase 1  router fp32 -> exact top-2 gates; per-tile PE transposes feed the
           router matmul; vals/gvals built vectorized after the tile loop.
  Index    fold [128,NT,2,E] -> 16-partition wrap (8 strided sb2sb DMAs),
           per-expert gpsimd sparse_gather (idx list + gate list), vectorized
           predication across all experts, two index lists per expert:
           gather-idx (pads->0) and scatter-idx = token + Bc*(second-choice)
           with pads -> dump row 2Bc. Replicated to 128 partitions.
  Phase 2  per expert: dma_gather x rows (transpose) -> FFN bf16 with
           host-prelayout contiguous weights -> relu*gate on ScalarE ->
           dma_scatter_add into moe12[rep%2]. First/second-choice halves of
           moe12 give disjoint rows per expert -> scatters need no ordering.
  Phase 3  per 512-token chunk: moeT = half1+half2 (DMA-transpose loads +
           DVE add), QKV (contiguous weights, bias via ScalarE), scores ->
           exp(RSQD*s) fused on ScalarE, head-sum via DVE, outT = expS*vT,
           out = (outT@wo)*(1/sum) + bo.
Cross-rep pipelining: per-rep state lives in bufs=2 pools; moe12 double-
buffered across reps so rep n+1's router/experts overlap rep n's MLA.
"""
import sys

sys.path.insert(0, "/opt/trn_rl_repo")

from contextlib import ExitStack

import numpy as np
import ml_dtypes

import concourse.bass as bass
import concourse.tile as tile
from concourse import bacc, mybir
from concourse.masks import make_identity
from concourse.tile import add_dep_helper as _adh


def add_dep(a, b, reason=""):
    ia = a.ins if hasattr(a, "ins") and not isinstance(a.ins, list) else a
    ib = b.ins if hasattr(b, "ins") and not isinstance(b.ins, list) else b
    _adh(ia, ib, reason=reason)

FP32 = mybir.dt.float32
BF16 = mybir.dt.bfloat16
I16 = mybir.dt.int16
I32 = mybir.dt.int32
U32 = mybir.dt.uint32
Alu = mybir.AluOpType
Act = mybir.ActivationFunctionType

F = 2048      # input feature dim
E = 16        # experts
U = 2048      # expert hidden dim
D = 2048      # d_model
H = 16        # heads
DEPTH = 128   # d_model // H
FT = F // 128   # 16 f-tiles
UT = U // 128   # 16 u-tiles
DT = D // 128   # 16 d-tiles
RSQD = 1.0 / float(np.sqrt(np.float32(DEPTH)))


def build(bc, sparse=True, cap=256, n_cores=8, debug=False, reps=1):
    Bc = bc
    NT = Bc // 128          # token tiles per core
    CT = cap // 128         # capacity tiles per expert
    NW = Bc // 16           # wrapped free dim per expert
    CW = cap // 16
    CH = min(512, Bc)       # MLA token chunk
    NCH = Bc // CH
    assert Bc % 128 == 0 and cap % 128 == 0

    nc = bacc.Bacc("TRN2", target_bir_lowering=False, debug=False,
                   num_devices=n_cores)

    # ---------------- DRAM tensors (host-prelayout, all loads contiguous) ---
    x_d = nc.dram_tensor("x", [Bc, F], FP32, kind="ExternalInput").ap()
    xb_d = nc.dram_tensor("x_bf16", [Bc, F], BF16, kind="ExternalInput").ap()
    # router_w host-permuted: rw2[p, ft, e] = router_w[ft*128+p, e]
    rw_d = nc.dram_tensor("router_w", [128, FT, E], FP32,
                          kind="ExternalInput").ap()
    rb_d = nc.dram_tensor("router_b", [1, E], FP32, kind="ExternalInput").ap()
    # expert_w host-permuted: w2[e, uc, p, ft, j] = w[e, ft*128+p, uc*512+j]
    w_d = nc.dram_tensor("expert_w", [E, U // 512, 128, FT, 512], BF16,
                         kind="ExternalInput").ap()
    eb_d = nc.dram_tensor("expert_b", [E, U], BF16, kind="ExternalInput").ap()
    # wq/wk/wv host-permuted: [dc, p, ut, j] = w[ut*128+p, dc*512+j]
    wq_d = nc.dram_tensor("wq", [D // 512, 128, UT, 512], BF16,
                          kind="ExternalInput").ap()
    wk_d = nc.dram_tensor("wk", [D // 512, 128, UT, 512], BF16,
                          kind="ExternalInput").ap()
    wv_d = nc.dram_tensor("wv", [D // 512, 128, UT, 512], BF16,
                          kind="ExternalInput").ap()
    # wo host-permuted: [dc, p, dt, j] = wo[dt*128+p, dc*512+j]
    wo_d = nc.dram_tensor("wo", [D // 512, 128, DT, 512], BF16,
                          kind="ExternalInput").ap()
    # qkv biases host-transposed: [p, dt] = b[dt*128+p]
    bq_d = nc.dram_tensor("bq", [128, DT], BF16, kind="ExternalInput").ap()
    bk_d = nc.dram_tensor("bk", [128, DT], BF16, kind="ExternalInput").ap()
    bv_d = nc.dram_tensor("bv", [128, DT], BF16, kind="ExternalInput").ap()
    bo_d = nc.dram_tensor("bo", [1, D], FP32, kind="ExternalInput").ap()
    # double-buffered combine target; rows [0,Bc) first-choice, [Bc,2Bc)
    # second-choice, row block [2Bc,2Bc+128) = dump rows for pad slots
    moe_d = [nc.dram_tensor(f"moe{i}", [2 * Bc + 128, U], BF16).ap()
             for i in range(2)]
    out_d = nc.dram_tensor("out", [Bc, D], FP32, kind="ExternalOutput").ap()

    with tile.TileContext(nc) as tc, ExitStack() as top:
        const = top.enter_context(tc.tile_pool(name="const", bufs=1))

        ident = const.tile([128, 128], FP32)
        make_identity(nc, ident)
        ident_bf = const.tile([128, 128], BF16)
        nc.vector.tensor_copy(ident_bf, ident)
        ones_sq = const.tile([128, 128], BF16)
        nc.vector.memset(ones_sq, 1.0)
        ztile = const.tile([128, U], BF16)
        nc.vector.memset(ztile, 0.0)

        rw_sb = const.tile([128, FT, E], FP32)
        nc.sync.dma_start(rw_sb, rw_d)
        rb_b = const.tile([128, E], FP32)
        nc.sync.dma_start(rb_b, rb_d.to_broadcast([128, E]))
        bqT = const.tile([128, DT], BF16)
        nc.sync.dma_start(bqT, bq_d)
        bkT = const.tile([128, DT], BF16)
        nc.sync.dma_start(bkT, bk_d)
        bvT = const.tile([128, DT], BF16)
        nc.sync.dma_start(bvT, bv_d)
        bo_b = const.tile([128, D], FP32)
        nc.sync.dma_start(bo_b, bo_d.to_broadcast([128, D]))
        eb16 = const.tile([16, U], BF16)
        nc.sync.dma_start(eb16, eb_d)

        # iotas: bp1[p, t] = t*128 + p + 1 (token id + 1);
        # slotpos[q, w] = w*16 + q (compressed-stream position)
        iot = const.tile([128, NT], I32)
        nc.gpsimd.iota(iot, pattern=[[128, NT]], base=1, channel_multiplier=1)
        bp1 = const.tile([128, NT], FP32)
        nc.vector.tensor_copy(bp1, iot)
        slotpos_i = const.tile([16, CW], I32)
        nc.gpsimd.iota(slotpos_i, pattern=[[16, CW]], base=0,
                       channel_multiplier=1)
        slotpos = const.tile([16, CW], FP32)
        nc.vector.tensor_copy(slotpos, slotpos_i)

        # ---- persistent pools (bufs chosen for cross-rep pipelining) ----
        state = top.enter_context(tc.tile_pool(name="state", bufs=2))
        idxp = top.enter_context(tc.tile_pool(name="idxp", bufs=1))
        idxp2 = top.enter_context(tc.tile_pool(name="idxp2", bufs=2))
        xpool = top.enter_context(tc.tile_pool(name="xload", bufs=2))
        xtp = top.enter_context(tc.tile_pool(name="xtp", bufs=3))
        sft = top.enter_context(tc.tile_pool(name="sft", bufs=2))
        gpool = top.enter_context(tc.tile_pool(name="gtiles", bufs=2))
        wpool = top.enter_context(tc.tile_pool(name="wtiles", bufs=2))
        ypool = top.enter_context(tc.tile_pool(name="ypool", bufs=2))
        mla = top.enter_context(tc.tile_pool(name="mla", bufs=1))
        mla2 = top.enter_context(tc.tile_pool(name="mla2", bufs=2))
        tpl = top.enter_context(tc.tile_pool(name="tpl", bufs=2))
        opool = top.enter_context(tc.tile_pool(name="osb", bufs=2))

        tp1 = top.enter_context(tc.tile_pool(name="tp1", bufs=2,
                                             space="PSUM"))
        rp = top.enter_context(tc.tile_pool(name="rp", bufs=1, space="PSUM"))
        ep = top.enter_context(tc.tile_pool(name="ep", bufs=2, space="PSUM"))
        mp = top.enter_context(tc.tile_pool(name="mp", bufs=2, space="PSUM"))
        tp3 = top.enter_context(tc.tile_pool(name="tp3", bufs=1,
                                             space="PSUM"))

        last_moeT_loads = [[], []]   # per moe buffer: loads to WAR against
        front = {}                   # rep -> dict with phase-1/index products

        # DMA queue split: SP (nc.sync) carries only dep-free prefetch
        # streams (x, weights); Activation hosts DMAs that legitimately
        # block (zeros, moeT transposes, out writes); Pool hosts the small
        # index-build DMAs so they only ever block the gathers behind them.

        def emit_front(rep):
            mb = rep % 2
            moe = moe_d[mb]

            # =========== Phase 1: router (fp32) + gates ===========
            gate_sb = state.tile([128, NT, E], FP32, tag="gate")
            mask_sb = state.tile([128, NT, E], FP32, tag="mask")
            mask1_sb = state.tile([128, NT, E], FP32, tag="mask1")
            valpack = state.tile([128, 2, E, NT], FP32, tag="valpack")

            for bt in range(NT):
                x_sb = xpool.tile([128, F], FP32, tag="x")
                nc.sync.dma_start(x_sb, x_d[bt * 128:(bt + 1) * 128, :])
                lp = rp.tile([128, E], FP32, tag="lp")
                for ft in range(FT):
                    pt = tp1.tile([128, 128], FP32, tag="pt")
                    nc.tensor.transpose(
                        pt, x_sb[:, ft * 128:(ft + 1) * 128], ident)
                    xT32 = xtp.tile([128, 128], FP32, tag="xT32")
                    nc.vector.tensor_copy(xT32, pt)
                    nc.tensor.matmul(lp, xT32, rw_sb[:, ft, :],
                                     start=(ft == 0), stop=(ft == FT - 1))
                lg = sft.tile([128, E], FP32, tag="lg")
                nc.vector.tensor_tensor(lg, lp, rb_b, Alu.add)
                top8 = sft.tile([128, 8], FP32, tag="top8")
                nc.vector.max(top8, lg)
                nc.vector.tensor_scalar(mask_sb[:, bt, :], lg, top8[:, 1:2],
                                        None, Alu.is_ge)
                nc.vector.tensor_scalar(mask1_sb[:, bt, :], lg, top8[:, 0:1],
                                        None, Alu.is_ge)
                ex = sft.tile([128, E], FP32, tag="ex")
                nc.vector.tensor_scalar(ex, lg, top8[:, 0:1], None,
                                        Alu.subtract)
                nc.scalar.activation(ex, ex, Act.Exp)
                ssum = sft.tile([128, 1], FP32, tag="ssum")
                nc.vector.reduce_sum(ssum, ex, mybir.AxisListType.X)
                rec = sft.tile([128, 1], FP32, tag="rec")
                nc.vector.reciprocal(rec, ssum)
                nc.vector.tensor_scalar(ex, ex, rec, None, Alu.mult)
                nc.vector.tensor_tensor(gate_sb[:, bt, :], ex,
                                        mask_sb[:, bt, :], Alu.mult)

            # zero the combine buffer (scatter_add needs += 0 semantics)
            zero_insts = []
            for t in range(2 * NT):
                z = nc.scalar.dma_start(moe[t * 128:(t + 1) * 128, :], ztile)
                for ld in last_moeT_loads[mb]:
                    add_dep(z, ld, reason="moe WAR across reps")
                zero_insts.append(z)
            last_moeT_loads[mb] = []

            # vals = mask*(b+1) - 1 + Bc*(mask - mask1); gvals = gate + mask-1
            vp0 = valpack[:, 0].rearrange("p e t -> p t e")
            vp1 = valpack[:, 1].rearrange("p e t -> p t e")
            nc.vector.tensor_tensor(
                vp0, mask_sb, bp1[:, :, None].to_broadcast([128, NT, E]),
                Alu.mult)
            nc.vector.tensor_scalar(vp0, vp0, 1.0, None, Alu.subtract)
            is2 = state.tile([128, NT, E], FP32, tag="is2")
            nc.vector.tensor_tensor(is2, mask_sb, mask1_sb, Alu.subtract)
            nc.vector.tensor_scalar(is2, is2, float(Bc), None, Alu.mult)
            nc.vector.tensor_tensor(vp0, vp0, is2, Alu.add)
            nc.vector.tensor_scalar(vp1, mask_sb, 1.0, None, Alu.subtract)
            nc.vector.tensor_tensor(vp1, vp1, gate_sb, Alu.add)

            # =========== Index build ===========
            # fold to 16-partition wrap: vg_w[q, c, e, s*NT+t] =
            #   valpack[16s+q, t, c, e]
            vg_w = idxp.tile([16, 2, E, NW], FP32, tag="vg_w")
            vg_flat = vg_w.rearrange("p c e w -> p (c e) w")
            for s in range(8):
                nc.gpsimd.dma_start(
                    vg_flat[:, :, s * NT:(s + 1) * NT],
                    valpack[16 * s:16 * (s + 1)].rearrange(
                        "p c e t -> p (c e) t"))

            iraw = idxp.tile([16, E, CW], FP32, tag="iraw")
            graw = idxp.tile([16, E, CW], FP32, tag="graw")
            nf_all = idxp.tile([1, 2, E], U32, tag="nf")
            for e in range(E):
                nc.gpsimd.sparse_gather(iraw[:, e, :], vg_w[:, 0, e, :],
                                        num_found=nf_all[:, 0, e:e + 1])
                nc.gpsimd.sparse_gather(graw[:, e, :], vg_w[:, 1, e, :],
                                        num_found=nf_all[:, 1, e:e + 1])

            # vectorized predication over all experts
            cntf = idxp.tile([1, E], FP32, tag="cntf")
            nc.vector.tensor_copy(cntf, nf_all[:, 0, :])
            cnt_b = idxp.tile([16, E], FP32, tag="cnt_b")
            nc.gpsimd.partition_broadcast(cnt_b, cntf)
            pmask = idxp.tile([16, E, CW], U32, tag="pmask")
            nc.vector.tensor_tensor(
                pmask, slotpos[:, None, :].to_broadcast([16, E, CW]),
                cnt_b[:, :, None].to_broadcast([16, E, CW]), Alu.is_lt)
            # gather idx: token id (strip the +Bc second-choice offset), pad->0
            ge2 = idxp.tile([16, E, CW], FP32, tag="ge2")
            nc.vector.tensor_scalar(ge2, iraw, float(Bc), None, Alu.is_ge)
            nc.vector.tensor_scalar(ge2, ge2, float(Bc), None, Alu.mult)
            nc.vector.tensor_tensor(ge2, iraw, ge2, Alu.subtract)
            idxf = idxp.tile([16, 2, E, CW], FP32, tag="idxf")
            nc.vector.memset(idxf[:, 0], 0.0)
            nc.vector.memset(idxf[:, 1], float(2 * Bc))
            nc.vector.copy_predicated(idxf[:, 0], pmask, ge2)
            nc.vector.copy_predicated(idxf[:, 1], pmask, iraw)
            g_all = idxp.tile([16, E, CW], FP32, tag="g_all")
            nc.vector.memset(g_all, 0.0)
            nc.vector.copy_predicated(g_all, pmask, graw)

            idx16 = idxp.tile([16, 2, E, CW], I16, tag="idx16")
            nc.vector.tensor_copy(idx16, idxf)
            idx_rep = idxp2.tile([128, 2, E, CW], I16, tag="idx_rep")
            nc.gpsimd.dma_start(idx_rep[0:16], idx16)
            nc.gpsimd.dma_start(idx_rep[16:32], idx_rep[0:16])
            nc.gpsimd.dma_start(idx_rep[32:64], idx_rep[0:32])
            nc.gpsimd.dma_start(idx_rep[64:128], idx_rep[0:64])
            # slot gates to slot-major: sg[16s+q, e, ct] = g[q, e, 8ct+s]
            slotg = idxp2.tile([128, E, CT], FP32, tag="slotg")
            gv = g_all.rearrange("p e (c s) -> p e c s", s=8)
            for s in range(8):
                nc.gpsimd.dma_start(slotg[16 * s:16 * (s + 1)],
                                    gv[:, :, :, s])

            front[rep] = dict(moe=moe, mb=mb, idx_rep=idx_rep, slotg=slotg,
                              zero_insts=zero_insts)

        def emit_experts(rep):
            fr = front[rep]
            moe, idx_rep, slotg = fr["moe"], fr["idx_rep"], fr["slotg"]
            zero_insts = fr["zero_insts"]
            # =========== Phase 2: expert FFN ===========
            scatters = []
            for e in range(E):
                xgT = gpool.tile([128, FT, cap], BF16, tag="xgT")
                nc.gpsimd.dma_gather(xgT, xb_d, idx_rep[:, 0, e, :],
                                     num_idxs=cap, num_idxs_reg=cap,
                                     elem_size=F, transpose=True)
                yb = ypool.tile([128, CT, U], BF16, tag="yb")
                for uc in range(U // 512):
                    wt = wpool.tile([128, FT, 512], BF16, tag="wt")
                    nc.sync.dma_start(wt, w_d[e, uc])
                    for ct in range(CT):
                        ps = ep.tile([128, 512], FP32, tag="eps")
                        for ft in range(FT):
                            nc.tensor.matmul(
                                ps, xgT[:, ft, ct * 128:(ct + 1) * 128],
                                wt[:, ft, :], start=(ft == 0), stop=False)
                        nc.tensor.matmul(
                            ps,
                            ident_bf[0:16, e:e + 1].to_broadcast([16, 128]),
                            eb16[:, uc * 512:(uc + 1) * 512],
                            start=False, stop=True)
                        nc.scalar.activation(
                            yb[:, ct, uc * 512:(uc + 1) * 512], ps, Act.Relu,
                            scale=slotg[:, e, ct:ct + 1])
                sc = nc.gpsimd.dma_scatter_add(moe, yb, idx_rep[:, 1, e, :],
                                               num_idxs=cap, num_idxs_reg=cap,
                                               elem_size=U)
                for z in zero_insts:
                    add_dep(sc, z, reason="moe zero->scatter")
                scatters.append(sc)
            return scatters

        def emit_mla(rep, scatters):
            fr = front[rep]
            moe, mb = fr["moe"], fr["mb"]
            # =========== Phase 3: MLA (per 512-token chunk) ===========
            rectok = mla2.tile([128, NT], FP32, tag="rectok")
            for ch in range(NCH):
                c0 = ch * CH
                moeT = mla.tile([128, UT, CH], BF16, tag="moeT")
                for ut in range(UT):
                    t1 = tpl.tile([128, CH], BF16, tag="t1")
                    ld1 = nc.scalar.dma_start_transpose(
                        t1, moe[c0:c0 + CH, ut * 128:(ut + 1) * 128])
                    t2 = tpl.tile([128, CH], BF16, tag="t2")
                    ld2 = nc.scalar.dma_start_transpose(
                        t2, moe[Bc + c0:Bc + c0 + CH,
                                ut * 128:(ut + 1) * 128])
                    for sc in scatters:
                        add_dep(ld1, sc, reason="moe RAW")
                        add_dep(ld2, sc, reason="moe RAW")
                    last_moeT_loads[mb].append(ld1)
                    last_moeT_loads[mb].append(ld2)
                    nc.vector.tensor_tensor(moeT[:, ut, :], t1, t2, Alu.add)

                vT = mla.tile([128, DT, CH], BF16, tag="vT")
                expS = mla.tile([128, H, CH], BF16, tag="expS")
                for dc in range(D // 512):
                    q4 = mla.tile([128, 4, CH], BF16, tag="q4")
                    k4 = mla.tile([128, 4, CH], BF16, tag="k4")
                    for (w_dram, bT, dst, base) in (
                            (wq_d, bqT, q4, None),
                            (wk_d, bkT, k4, None),
                            (wv_d, bvT, vT, dc * 4)):
                        wt = wpool.tile([128, UT, 512], BF16, tag="wt")
                        nc.sync.dma_start(wt, w_dram[dc])
                        for sub in range(4):
                            dt = dc * 4 + sub
                            ps = mp.tile([128, CH], FP32, tag="mla_ps")
                            for ut in range(UT):
                                nc.tensor.matmul(
                                    ps, wt[:, ut, sub * 128:(sub + 1) * 128],
                                    moeT[:, ut, :],
                                    start=(ut == 0), stop=(ut == UT - 1))
                            di = sub if base is None else dc * 4 + sub
                            nc.scalar.activation(
                                dst[:, di, :], ps, Act.Identity,
                                bias=bT[:, dt:dt + 1])
                    # scores for these 4 heads: s = sum_d q*k; expS=exp(s/sqrt)
                    nc.vector.tensor_tensor(q4, q4, k4, Alu.mult)
                    for h4 in range(4):
                        psS = mp.tile([128, CH], FP32, tag="mla_ps")
                        nc.tensor.matmul(psS, ones_sq, q4[:, h4, :],
                                         start=True, stop=True)
                        nc.scalar.activation(expS[:, dc * 4 + h4, :], psS,
                                             Act.Exp, scale=RSQD)

                # softmax denominator and combine
                Ss = mla.tile([128, CH], FP32, tag="Ss")
                nc.vector.reduce_sum(Ss, expS.rearrange("p h b -> p b h"),
                                     mybir.AxisListType.X)
                outT = mla.tile([128, DT, CH], BF16, tag="outT")
                nc.vector.tensor_tensor(outT, expS, vT, Alu.mult)
                for bt4 in range(CH // 128):
                    pt = tp3.tile([128, 128], FP32, tag="pt3")
                    nc.tensor.transpose(
                        pt, Ss[:, bt4 * 128:(bt4 + 1) * 128], ident)
                    nc.vector.tensor_copy(
                        rectok[:, ch * (CH // 128) + bt4:
                               ch * (CH // 128) + bt4 + 1], pt[:, 0:1])
                nc.vector.reciprocal(
                    rectok[:, ch * (CH // 128):(ch + 1) * (CH // 128)],
                    rectok[:, ch * (CH // 128):(ch + 1) * (CH // 128)])

                # out = (outT.T @ wo) * rectok + bo
                for dct in range(D // 512):
                    wo_sb = wpool.tile([128, DT, 512], BF16, tag="wt")
                    nc.sync.dma_start(wo_sb, wo_d[dct])
                    for bt4 in range(CH // 128):
                        bt = ch * (CH // 128) + bt4
                        ps = mp.tile([128, 512], FP32, tag="mla_ps")
                        for dt in range(DT):
                            nc.tensor.matmul(
                                ps, outT[:, dt, bt4 * 128:(bt4 + 1) * 128],
                                wo_sb[:, dt, :],
                                start=(dt == 0), stop=(dt == DT - 1))
                        o_sb = opool.tile([128, 512], FP32, tag="o_sb")
                        nc.scalar.activation(o_sb, ps, Act.Copy,
                                             scale=rectok[:, bt:bt + 1])
                        nc.vector.tensor_tensor(
                            o_sb, o_sb, bo_b[:, dct * 512:(dct + 1) * 512],
                            Alu.add)
                        nc.scalar.dma_start(
                            out_d[bt * 128:(bt + 1) * 128,
                                  dct * 512:(dct + 1) * 512], o_sb)

        # software-pipelined emission: phase1+index of rep r+1 lands between
        # phase 2 and phase 3 of rep r on every engine queue.
        emit_front(0)
        for rep in range(reps):
            scatters = emit_experts(rep)
            if rep + 1 < reps:
                emit_front(rep + 1)
            emit_mla(rep, scatters)

    nc.compile()
    return nc


# ---------------------------------------------------------------------------
# Self-contained entry point: kernel(**inputs) -> np.ndarray  [8192, 2048] f32

N_CORES = 8
BC = 1024          # tokens per core (B = 8192)
CAP = 256          # per-expert per-core capacity (>= observed max 155)

_nc_cache = {}


def _get_nc():
    if "nc" not in _nc_cache:
        _nc_cache["nc"] = build(BC, sparse=True, cap=CAP, n_cores=N_CORES)
    return _nc_cache["nc"]


def _make_in_maps(inputs):
    bf = ml_dtypes.bfloat16
    w = np.asarray(inputs["expert_w"], dtype=np.float32)
    w_bf = np.ascontiguousarray(
        w.reshape(E, FT, 128, U // 512, 512).transpose(0, 3, 2, 1, 4)
    ).astype(bf)
    def qkvperm(a):
        a = np.asarray(a, dtype=np.float32)
        return np.ascontiguousarray(
            a.reshape(UT, 128, D // 512, 512).transpose(2, 1, 0, 3)
        ).astype(bf)
    wq_bf = qkvperm(inputs["wq"])
    wk_bf = qkvperm(inputs["wk"])
    wv_bf = qkvperm(inputs["wv"])
    wo_bf = qkvperm(inputs["wo"])
    eb_bf = np.ascontiguousarray(inputs["expert_b"]).astype(bf)
    rw2 = np.ascontiguousarray(
        np.asarray(inputs["router_w"], dtype=np.float32)
        .reshape(FT, 128, E).transpose(1, 0, 2))
    def bT(b):
        return np.ascontiguousarray(
            np.asarray(b, dtype=np.float32).reshape(DT, 128).T).astype(bf)
    in_maps = []
    for c in range(N_CORES):
        xs = np.ascontiguousarray(
            np.asarray(inputs["x"])[c * BC:(c + 1) * BC]).astype(np.float32)
        m = {
            "x": xs,
            "x_bf16": xs.astype(bf),
            "router_w": rw2,
            "router_b": np.asarray(
                inputs["router_b"], dtype=np.float32).reshape(1, E),
            "expert_w": w_bf,
            "expert_b": eb_bf,
            "wq": wq_bf, "wk": wk_bf, "wv": wv_bf, "wo": wo_bf,
            "bq": bT(inputs["bq"]), "bk": bT(inputs["bk"]),
            "bv": bT(inputs["bv"]),
            "bo": np.asarray(inputs["bo"], dtype=np.float32).reshape(1, D),
        }
        in_maps.append(m)
    return in_maps


def kernel(**inputs):
    from concourse.bass_utils import run_bass_kernel_spmd
    nc = _get_nc()
    in_maps = _make_in_maps(inputs)
    res = run_bass_kernel_spmd(nc, in_maps, core_ids=list(range(N_CORES)))
    out = np.concatenate([res.results[c]["out"] for c in range(N_CORES)],
                         axis=0)
    return np.ascontiguousarray(out.astype(np.float32))
